# revision 2
# baseline (speedup 1.0000x reference)
"""Trainium2 Bass kernel for nn_DHSLayer (DHS-pruned ViT attention layer).

Strategy: data-parallel over batch (B=128 -> 16 per core x 8 cores).
All matmuls in fp16 (fp32 PSUM accumulation); softmax without max-subtraction
(scores are O(1) here); denominator folded into the ctx matmul via a ones
column appended to V; residual folded into the dense weight (We = Wd + I);
V bias folded into the dense bias (softmax rows sum to 1).

Self-contained: hardcodes shapes B=128, S=197, D=768, H=12, DH=64.
"""

import os
import sys
from contextlib import ExitStack

import numpy as np

for _p in ("/opt/trn_rl_repo", "/root/.axon_site/_ro/trn_rl_repo"):
    if os.path.isdir(_p) and _p not in sys.path:
        sys.path.append(_p)

import concourse.bass as bass
import concourse.tile as tile
from concourse import bacc, mybir
from concourse import bass_utils
from concourse.masks import make_identity

F16 = mybir.dt.float16
F32 = mybir.dt.float32
AF = mybir.ActivationFunctionType
ALU = mybir.AluOpType

D = 768
S = 197
NH = 12
DH = 64
NCORES = 8
DT = 6  # number of 128-wide d tiles
TT = ((0, 128), (128, 69))  # token tiles covering S=197
# sigmoid(z) >= 0.05  <=>  z >= log(0.05/0.95)
LOGIT_THR = -2.9444389791664403


def _body(ctx, tc, io, n_b):
    nc = tc.nc
    const = ctx.enter_context(tc.tile_pool(name="const", bufs=1))
    p_in = ctx.enter_context(tc.tile_pool(name="p_in", bufs=2))
    p_x16 = ctx.enter_context(tc.tile_pool(name="p_x16", bufs=2))
    p_qk = ctx.enter_context(tc.tile_pool(name="p_qk", bufs=2))
    p_v = ctx.enter_context(tc.tile_pool(name="p_v", bufs=2))
    p_mlp = ctx.enter_context(tc.tile_pool(name="p_mlp", bufs=2))
    p_att = ctx.enter_context(tc.tile_pool(name="p_att", bufs=4))
    p_ctx = ctx.enter_context(tc.tile_pool(name="p_ctx", bufs=2))
    p_sel = ctx.enter_context(tc.tile_pool(name="p_sel", bufs=2))
    ps = ctx.enter_context(tc.tile_pool(name="ps", bufs=8, space="PSUM"))

    def pst(shape, dtype=F32):
        return ps.tile(shape, dtype, tag="ps", name="pst")

    # ---- constants ----
    ident16 = const.tile([128, 128], F16)
    make_identity(nc, ident16)
    ones16 = const.tile([1, 1], F16)
    nc.vector.memset(ones16, 1.0)

    wq16 = const.tile([128, DT * D], F16)
    nc.sync.dma_start(wq16.rearrange("p (j n) -> p j n", j=DT), io["wq"].rearrange("(j p) n -> p j n", p=128))
    wk16 = const.tile([128, DT * D], F16)
    nc.sync.dma_start(wk16.rearrange("p (j n) -> p j n", j=DT), io["wk"].rearrange("(j p) n -> p j n", p=128))
    wv16 = const.tile([128, DT * D], F16)
    nc.sync.dma_start(wv16.rearrange("p (j n) -> p j n", j=DT), io["wv"].rearrange("(j p) n -> p j n", p=128))
    we16 = const.tile([128, DT * D], F16)
    nc.sync.dma_start(we16.rearrange("p (j n) -> p j n", j=DT), io["we"].rearrange("(j p) n -> p j n", p=128))
    w116 = const.tile([128, DT * 64], F16)
    nc.sync.dma_start(w116.rearrange("p (j n) -> p j n", j=DT), io["w1"].rearrange("(j p) n -> p j n", p=128))
    w216 = const.tile([64, 1], F16)
    nc.sync.dma_start(w216, io["w2"])

    bqs32 = const.tile([128, DT], F32)
    nc.sync.dma_start(bqs32, io["bqs"])
    bks32 = const.tile([128, DT], F32)
    nc.sync.dma_start(bks32, io["bks"])
    b1c32 = const.tile([64, 1], F32)
    nc.sync.dma_start(b1c32, io["b1c"])
    bde32 = const.tile([128, D], F32)
    bde_bcast = bass.AP(
        tensor=io["bde"].tensor,
        offset=io["bde"].offset,
        ap=[[0, 128], [1, D]],
    )
    nc.sync.dma_start(bde32, bde_bcast)

    hs = io["hs"]
    out = io["out"]

    for b in range(n_b):
        # ---- load X (token-major), cast fp16, transpose to feature-major ----
        xtm32_a = p_in.tile([128, D], F32)
        nc.sync.dma_start(xtm32_a, hs[b, 0:128, :])
        xtm32_b = p_in.tile([69, D], F32)
        nc.sync.dma_start(xtm32_b, hs[b, 128:197, :])

        xtm16_a = p_x16.tile([128, D], F16)
        nc.vector.tensor_copy(xtm16_a, xtm32_a)
        xtm16_b = p_x16.tile([69, D], F16)
        nc.vector.tensor_copy(xtm16_b, xtm32_b)

        xfm16 = p_x16.tile([128, DT * S], F16)
        for j in range(DT):
            for (t0, tsz), xtm in zip(TT, (xtm16_a, xtm16_b)):
                tp = pst([128, tsz], F16)
                nc.tensor.transpose(
                    tp, xtm[0:tsz, j * 128 : (j + 1) * 128], ident16[0:tsz, 0:tsz]
                )
                nc.vector.tensor_copy(xfm16[:, j * S + t0 : j * S + t0 + tsz], tp)

        # ---- Q/K projections (feature-major out) ----
        q16 = p_qk.tile([128, DT * S], F16)
        k16 = p_qk.tile([128, DT * S], F16)
        for jo in range(DT):
            qp = pst([128, S])
            for ji in range(DT):
                nc.tensor.matmul(
                    qp,
                    lhsT=wq16[:, ji * D + jo * 128 : ji * D + (jo + 1) * 128],
                    rhs=xfm16[:, ji * S : (ji + 1) * S],
                    start=(ji == 0),
                    stop=(ji == DT - 1),
                )
            nc.scalar.activation(
                q16[:, jo * S : (jo + 1) * S],
                qp,
                AF.Identity,
                bias=bqs32[:, jo : jo + 1],
                scale=0.125,
            )
            kp = pst([128, S])
            for ji in range(DT):
                nc.tensor.matmul(
                    kp,
                    lhsT=wk16[:, ji * D + jo * 128 : ji * D + (jo + 1) * 128],
                    rhs=xfm16[:, ji * S : (ji + 1) * S],
                    start=(ji == 0),
                    stop=(ji == DT - 1),
                )
            nc.scalar.activation(
                k16[:, jo * S : (jo + 1) * S],
                kp,
                AF.Identity,
                bias=bks32[:, jo : jo + 1],
                scale=1.0,
            )

        # ---- V projection (token-major out, ones column appended per head) ----
        va = p_v.tile([128, NH, DH + 1], F16)
        vb = p_v.tile([69, NH, DH + 1], F16)
        nc.vector.memset(va[:, :, DH : DH + 1], 1.0)
        nc.vector.memset(vb[:, :, DH : DH + 1], 1.0)
        for (t0, tsz), vt in zip(TT, (va, vb)):
            for half in range(2):
                vp = pst([128, 384])
                for ji in range(DT):
                    nc.tensor.matmul(
                        vp[0:tsz, :],
                        lhsT=xfm16[:, ji * S + t0 : ji * S + t0 + tsz],
                        rhs=wv16[:, ji * D + half * 384 : ji * D + (half + 1) * 384],
                        start=(ji == 0),
                        stop=(ji == DT - 1),
                    )
                nc.vector.tensor_copy(
                    vt[0:tsz, half * 6 : (half + 1) * 6, 0:DH],
                    vp[0:tsz, :].rearrange("p (h d) -> p h d", h=6),
                )

        # ---- DHS scoring MLP -> keep mask (token-major columns) ----
        hp = pst([64, S])
        for ji in range(DT):
            nc.tensor.matmul(
                hp,
                lhsT=w116[:, ji * 64 : (ji + 1) * 64],
                rhs=xfm16[:, ji * S : (ji + 1) * S],
                start=(ji == 0),
                stop=(ji == DT - 1),
            )
        h116 = p_mlp.tile([64, S], F16)
        nc.scalar.activation(h116, hp, AF.Relu, bias=b1c32, scale=1.0)
        lp = pst([1, S])
        nc.tensor.matmul(lp, lhsT=w216, rhs=h116, start=True, stop=True)
        m16 = p_mlp.tile([1, S], F16)
        nc.vector.tensor_scalar(m16, lp, float(io["thr"]), None, op0=ALU.is_ge)
        nc.vector.memset(m16[0:1, 0:1], 1.0)  # CLS always kept

        mca = p_sel.tile([128, 1], F32)
        mcb = p_sel.tile([69, 1], F32)
        for (t0, tsz), mc in zip(TT, (mca, mcb)):
            mp = pst([tsz, 1])
            nc.tensor.matmul(
                mp, lhsT=m16[0:1, t0 : t0 + tsz], rhs=ones16, start=True, stop=True
            )
            nc.vector.tensor_copy(mc[0:tsz, :], mp)

        # ---- attention (scores transposed; ctx token-major) ----
        ca = p_ctx.tile([128, D], F16)
        cb = p_ctx.tile([69, D], F16)
        for h in range(NH):
            pb = (h % 2) * 64
            hj = h // 2
            expts = []
            for kt0, ksz in TT:
                sp = pst([128, S])
                nc.tensor.matmul(
                    sp[0:ksz, :],
                    lhsT=k16[pb : pb + 64, hj * S + kt0 : hj * S + kt0 + ksz],
                    rhs=q16[pb : pb + 64, hj * S : (hj + 1) * S],
                    start=True,
                    stop=True,
                )
                expt = p_att.tile([128, S], F16, tag="expt")
                nc.scalar.activation(expt[0:ksz, :], sp[0:ksz, :], AF.Exp)
                expts.append(expt)
            for (q0, qsz), ct in zip(TT, (ca, cb)):
                cp = pst([128, DH + 1])
                for i, (kt0, ksz) in enumerate(TT):
                    vt = va if i == 0 else vb
                    nc.tensor.matmul(
                        cp[0:qsz, :],
                        lhsT=expts[i][0:ksz, q0 : q0 + qsz],
                        rhs=vt[0:ksz, h, :],
                        start=(i == 0),
                        stop=(i == 1),
                    )
                rc = p_att.tile([128, 1], F32, tag="rc")
                nc.vector.reciprocal(rc[0:qsz, :], cp[0:qsz, DH : DH + 1])
                nc.vector.tensor_scalar(
                    ct[0:qsz, h * DH : (h + 1) * DH],
                    cp[0:qsz, 0:DH],
                    rc[0:qsz, 0:1],
                    None,
                    op0=ALU.mult,
                )

        # ---- transpose ctx to feature-major ----
        ctxf16 = p_ctx.tile([128, DT * S], F16)
        for j in range(DT):
            for (t0, tsz), ct in zip(TT, (ca, cb)):
                tp2 = pst([128, tsz], F16)
                nc.tensor.transpose(
                    tp2, ct[0:tsz, j * 128 : (j + 1) * 128], ident16[0:tsz, 0:tsz]
                )
                nc.vector.tensor_copy(ctxf16[:, j * S + t0 : j * S + t0 + tsz], tp2)

        # ---- dense (+folded residual/bias) and mask select, token-major ----
        for (t0, tsz), x32, mc in zip(TT, (xtm32_a, xtm32_b), (mca, mcb)):
            om = p_sel.tile([128, 1], F32, tag="om")
            nc.vector.tensor_scalar(
                om[0:tsz, :], mc[0:tsz, :], -1.0, 1.0, op0=ALU.mult, op1=ALU.add
            )
            t1 = p_sel.tile([128, D], F32, tag="t1")
            nc.vector.tensor_scalar(
                t1[0:tsz, :], x32[0:tsz, :], om[0:tsz, 0:1], None, op0=ALU.mult
            )
            z = p_sel.tile([128, D], F32, tag="z")
            nc.vector.scalar_tensor_tensor(
                z[0:tsz, :],
                bde32[0:tsz, :],
                mc[0:tsz, 0:1],
                t1[0:tsz, :],
                op0=ALU.mult,
                op1=ALU.add,
            )
            o32 = p_sel.tile([128, D], F32, tag="o32")
            for half in range(2):
                ap_ = pst([128, 384])
                for ji in range(DT):
                    nc.tensor.matmul(
                        ap_[0:tsz, :],
                        lhsT=ctxf16[:, ji * S + t0 : ji * S + t0 + tsz],
                        rhs=we16[:, ji * D + half * 384 : ji * D + (half + 1) * 384],
                        start=(ji == 0),
                        stop=(ji == DT - 1),
                    )
                nc.vector.scalar_tensor_tensor(
                    o32[0:tsz, half * 384 : (half + 1) * 384],
                    ap_[0:tsz, :],
                    mc[0:tsz, 0:1],
                    z[0:tsz, half * 384 : (half + 1) * 384],
                    op0=ALU.mult,
                    op1=ALU.add,
                )
            nc.sync.dma_start(out[b, t0 : t0 + tsz, :], o32[0:tsz, :])


def build_nc(n_b, thr):
    nc = bacc.Bacc(
        "TRN2", target_bir_lowering=False, debug=False, num_devices=NCORES
    )
    io = {
        "hs": nc.dram_tensor("hs", [n_b, S, D], F32, kind="ExternalInput").ap(),
        "wq": nc.dram_tensor("wq", [D, D], F16, kind="ExternalInput").ap(),
        "wk": nc.dram_tensor("wk", [D, D], F16, kind="ExternalInput").ap(),
        "wv": nc.dram_tensor("wv", [D, D], F16, kind="ExternalInput").ap(),
        "we": nc.dram_tensor("we", [D, D], F16, kind="ExternalInput").ap(),
        "w1": nc.dram_tensor("w1", [D, 64], F16, kind="ExternalInput").ap(),
        "w2": nc.dram_tensor("w2", [64, 1], F16, kind="ExternalInput").ap(),
        "bqs": nc.dram_tensor("bqs", [128, DT], F32, kind="ExternalInput").ap(),
        "bks": nc.dram_tensor("bks", [128, DT], F32, kind="ExternalInput").ap(),
        "b1c": nc.dram_tensor("b1c", [64, 1], F32, kind="ExternalInput").ap(),
        "bde": nc.dram_tensor("bde", [D], F32, kind="ExternalInput").ap(),
        "out": nc.dram_tensor("out", [n_b, S, D], F32, kind="ExternalOutput").ap(),
        "thr": thr,
    }
    with tile.TileContext(nc) as tc, ExitStack() as ctx:
        _body(ctx, tc, io, n_b)
    nc.compile()
    return nc


def make_host_inputs(Wq, bq, Wk, bk, Wv, bv, Wd, bd, W1, b1, W2, b2):
    """Host-side weight prep shared by all cores."""
    f32 = np.float32
    Wd = np.asarray(Wd, f32)
    bv = np.asarray(bv, f32)
    bd = np.asarray(bd, f32)
    we = Wd + np.eye(D, dtype=f32)
    bde = (bv @ we + bd).astype(f32)
    return {
        "wq": np.ascontiguousarray(np.asarray(Wq, f32).astype(np.float16)),
        "wk": np.ascontiguousarray(np.asarray(Wk, f32).astype(np.float16)),
        "wv": np.ascontiguousarray(np.asarray(Wv, f32).astype(np.float16)),
        "we": np.ascontiguousarray(we.astype(np.float16)),
        "w1": np.ascontiguousarray(np.asarray(W1, f32).astype(np.float16)),
        "w2": np.ascontiguousarray(
            np.asarray(W2, f32).astype(np.float16).reshape(64, 1)
        ),
        "bqs": np.ascontiguousarray(
            (np.asarray(bq, f32) / 8.0).reshape(DT, 128).T
        ),
        "bks": np.ascontiguousarray(np.asarray(bk, f32).reshape(DT, 128).T),
        "b1c": np.ascontiguousarray(np.asarray(b1, f32).reshape(64, 1)),
        "bde": bde,
    }, float(LOGIT_THR - float(np.asarray(b2, f32).reshape(-1)[0]))


_NC_CACHE = {}


def kernel(hidden_states, Wq, bq, Wk, bk, Wv, bv, Wd, bd, W1, b1, W2, b2):
    hs = np.ascontiguousarray(np.asarray(hidden_states, np.float32))
    B = hs.shape[0]
    n_b = B // NCORES
    weights, thr = make_host_inputs(Wq, bq, Wk, bk, Wv, bv, Wd, bd, W1, b1, W2, b2)

    key = (n_b, thr)
    if key not in _NC_CACHE:
        _NC_CACHE[key] = build_nc(n_b, thr)
    nc = _NC_CACHE[key]

    in_maps = [
        {**weights, "hs": np.ascontiguousarray(hs[c * n_b : (c + 1) * n_b])}
        for c in range(NCORES)
    ]
    res = bass_utils.run_bass_kernel_spmd(nc, in_maps, core_ids=list(range(NCORES)))
    return np.concatenate(
        [res.results[c]["out"] for c in range(NCORES)], axis=0
    ).astype(np.float32)
